# revision 12
# baseline (speedup 1.0000x reference)
"""Causal dilated 1D conv (KW=4, dilation=8) as shifted matmuls on 8 TRN2 cores.

out[b,o,t] = sum_{k,c} W[o, c*4+k] * x[b, c, t + k*8 - 24]

Sharding: data-parallel over batch (16 batches -> 2 per core). Each core runs
an identical program: all weights stationary in SBUF, x streamed in 512-wide
time blocks (+24 halo), 16 accumulating matmuls (4 c-chunks x 4 taps) per
(out-chunk, time-block) PSUM group, PSUM copied back via DVE and DMA'd out.

Matmuls run in bfloat16 (fp32 PSUM accumulate): 1 cycle/row streaming, and
unlike fp32/f32r the compiler-automatic Fast Weight Load path is enabled, so
the per-matmul LDWEIGHTS (97ns) hides under the previous matmul's 512-row
stream; measured steady-state cadence 216ns/MM = 512rows/2.4GHz + ~3ns NX
issue overhead, which is the HW floor (443us for 2048 MMs/core). bf16
quantization of x and W gives ~2.3e-3 relative error over the K=2048
contraction (gate 2e-2; fp8 DoubleRow measured 4e-2 on this data).

Edge optimizations (the steady state has no gaps; the NEFF preamble ~8us
and epilogue ~5us incl. a fixed 257-semaphore reset are not controllable):
- Warm-up MMs on a zeroed scratch tile keep the PE busy from the preamble
  barrier on, so the HAM activity monitor unthrottles the PE clock
  (1.2->2.4GHz after ~3.4us of sustained activity) during the DMA bootstrap
  and the first real MMs go warm as early as possible.
- x is pre-tiled on the host (halo duplicated) so every x-chunk DMA is one
  contiguous 137KB descriptor instead of 128 strided 1KB rows - the DGE
  descriptor generator, not the wire, limits bootstrap arrival.
- DMA is split across both hardware DGE queues (sync/SP + scalar/Act),
  interleaved so the first PSUM group's inputs lead both wire streams;
  steady-state output writes ride the scalar queue, inputs the sync queue.
- The very last PSUM group's drain is split across DVE+Act copies and both
  DMA queues (reusing the same tile, no extra semaphores) because the
  epilogue's reset sequence only starts once the final out-DMA completes.
"""

import ml_dtypes
import numpy as np

B = 16
C_IN = 512
C_OUT = 512
T = 8192
KW = 4
DIL = 8
PAD = (KW - 1) * DIL  # 24

N_CORES = 8
B_PER = B // N_CORES  # 2
P = 128
TBLK = 512
XW = TBLK + PAD       # 536
NT = T // TBLK        # 16
NCC = C_IN // P       # 4
NOC = C_OUT // P      # 4
N_WARM = 6

_cache = {}


def _build():
    import concourse.tile as tile
    from concourse import bacc, mybir

    nc = bacc.Bacc("TRN2", target_bir_lowering=False, debug=False,
                   num_devices=N_CORES)
    # x pre-tiled on host: [b, tb, cc, c=128, t=536] (halo duplicated)
    x = nc.dram_tensor("x", [B_PER, NT, NCC, P, XW], mybir.dt.bfloat16,
                       kind="ExternalInput").ap()
    # weights pre-arranged on host as [cc, tap, c=128, o=512]
    wt = nc.dram_tensor("wt", [NCC, KW, P, C_OUT], mybir.dt.bfloat16,
                        kind="ExternalInput").ap()
    out = nc.dram_tensor("out", [B_PER, C_OUT, T], mybir.dt.float32,
                         kind="ExternalOutput").ap()
    f32 = mybir.dt.float32
    bf16 = mybir.dt.bfloat16

    with tile.TileContext(nc) as tc:
        with tc.tile_pool(name="wpool", bufs=1) as wpool, \
             tc.tile_pool(name="xpool", bufs=4) as xpool, \
             tc.tile_pool(name="opool", bufs=4) as opool, \
             tc.tile_pool(name="pspool", bufs=4, space="PSUM") as pspool:

            # PE warm-up: zero a scratch tile and issue dependency-free MMs
            # so the PE is busy (and the HAM clock warming) during the DMA
            # bootstrap below.
            warm = wpool.tile([P, P + TBLK], bf16, name="warm", tag="warm")
            nc.vector.memset(warm[:], 0)
            wps = pspool.tile([P, TBLK], f32, name="ps", tag="ps")
            for _ in range(N_WARM):
                nc.tensor.matmul(wps[:], warm[:, :P], warm[:, P:],
                                 start=True, stop=True)

            def load_xt(b, tb, eng=None, ccs=range(NCC)):
                # One tile holds all 4 c-chunks side by side; per-chunk DMAs
                # keep arrival granularity via subtile deps.
                xt = xpool.tile([P, NCC * XW], bf16, name="xt", tag="xt")
                for cc in ccs:
                    (eng or nc.sync).dma_start(
                        xt[:, cc * XW:(cc + 1) * XW], x[b, tb, cc])
                return xt

            def rhs(xt, cc, k, o0=0, w=TBLK):
                o = cc * XW + o0 + k * DIL
                return xt[:, o: o + w]

            def lhsT(wt_cc, k, oc):
                o = k * C_OUT + oc * P
                return wt_cc[:, o: o + P]

            wtiles = [wpool.tile([P, KW * C_OUT], bf16, name=f"w{cc}",
                                 tag=f"w{cc}") for cc in range(NCC)]

            def load_w(cc, k, eng):
                eng.dma_start(wtiles[cc][:, k * C_OUT:(k + 1) * C_OUT],
                              wt[cc, k])

            # Bootstrap: the first PSUM group consumes (cc=0,k=0..3) first,
            # so xt0-chunk0 + w0 chunks lead the sync stream while the
            # scalar queue carries the next-needed chunks in parallel.
            xt0 = load_xt(0, 0, ccs=())
            nc.sync.dma_start(xt0[:, 0:XW], x[0, 0, 0])
            for k in range(KW):
                load_w(0, k, nc.sync)
            for cc in (1, 2, 3):
                nc.scalar.dma_start(xt0[:, cc * XW:(cc + 1) * XW],
                                    x[0, 0, cc])
            for k in range(KW):
                load_w(1, k, nc.scalar)
            for k in range(KW):
                load_w(2, k, nc.sync)
            for k in range(KW):
                load_w(3, k, nc.scalar)

            n_acc = NCC * KW
            cks = [(cc, k) for cc in range(NCC) for k in range(KW)]

            # Bootstrap block: emit MMs in weight-DMA-arrival order, fanning
            # each arriving weight chunk across the 4 oc PSUM banks, so the
            # in-order PE stream is never head-of-line blocked on a later
            # weight chunk.
            pss0 = [pspool.tile([P, TBLK], f32, name="ps", tag="ps")
                    for _ in range(NOC)]
            for ci, (cc, k) in enumerate(cks):
                for oc in range(NOC):
                    nc.tensor.matmul(
                        pss0[oc][:],
                        lhsT(wtiles[cc], k, oc),
                        rhs(xt0, cc, k),
                        start=(ci == 0),
                        stop=(ci == n_acc - 1),
                    )
            for oc in range(NOC):
                ot = opool.tile([P, TBLK], f32, name="ot", tag="ot")
                nc.vector.tensor_copy(ot[:], pss0[oc][:])
                nc.scalar.dma_start(out[0, oc * P:(oc + 1) * P, 0:TBLK],
                                    ot[:])

            H = TBLK // 2
            for b in range(B_PER):
                for tb in range(NT):
                    if b == 0 and tb == 0:
                        continue
                    final = (b == B_PER - 1 and tb == NT - 1)
                    xt = load_xt(b, tb)
                    for oc in range(NOC):
                        ps = pspool.tile([P, TBLK], f32, name="ps", tag="ps")
                        for ci, (cc, k) in enumerate(cks):
                            nc.tensor.matmul(
                                ps[:],
                                lhsT(wtiles[cc], k, oc),
                                rhs(xt, cc, k),
                                start=(ci == 0),
                                stop=(ci == n_acc - 1),
                            )
                        ot = opool.tile([P, TBLK], f32, name="ot", tag="ot")
                        orow = out[b, oc * P:(oc + 1) * P,
                                   tb * TBLK:(tb + 1) * TBLK]
                        if final and oc == NOC - 1:
                            # Split the very last drain across both copy
                            # engines and both DMA queues: the epilogue's
                            # fixed reset sequence starts only after the
                            # final out-DMA completes.
                            nc.vector.tensor_copy(ot[:, :H], ps[:, :H])
                            nc.sync.dma_start(orow[:, :H], ot[:, :H])
                            nc.scalar.copy(ot[:, H:], ps[:, H:])
                            nc.scalar.dma_start(orow[:, H:], ot[:, H:])
                        else:
                            nc.vector.tensor_copy(ot[:], ps[:])
                            nc.scalar.dma_start(orow, ot[:])

    nc.compile()
    return nc


def _get_nc():
    if "nc" not in _cache:
        _cache["nc"] = _build()
    return _cache["nc"]


def _make_in_maps(x, W):
    xb = np.ascontiguousarray(x, dtype=np.float32).astype(ml_dtypes.bfloat16)
    xpad = np.pad(xb, ((0, 0), (0, 0), (PAD, 0)))  # [B, C_IN, T+PAD]
    # pre-tile: xtl[b, tb, cc, c, j] = xpad[b, cc*128+c, tb*512 + j]
    sb, sc, st = xpad.strides
    xtv = np.lib.stride_tricks.as_strided(
        xpad, shape=(B, NT, NCC, P, XW),
        strides=(sb, st * TBLK, sc * P, sc, st))
    xtl = np.ascontiguousarray(xtv)
    w = np.ascontiguousarray(W, dtype=np.float32).reshape(C_OUT, C_IN, KW)
    # wt[cc, k, c, o] = W[o, (cc*128+c)*KW + k]
    wt = np.transpose(w.reshape(C_OUT, NCC, P, KW),
                      (1, 3, 2, 0)).astype(ml_dtypes.bfloat16).copy()
    return [{"x": np.ascontiguousarray(xtl[i * B_PER:(i + 1) * B_PER]),
             "wt": wt} for i in range(N_CORES)]


def kernel(x, W):
    from concourse.bass_utils import run_bass_kernel_spmd

    nc = _get_nc()
    in_maps = _make_in_maps(x, W)
    res = run_bass_kernel_spmd(nc, in_maps, list(range(N_CORES)))
    return np.concatenate([r["out"] for r in res.results], axis=0)


# revision 14
# speedup vs baseline: 1.0054x; 1.0054x over previous
"""Causal dilated 1D conv (KW=4, dilation=8) as shifted matmuls on 8 TRN2 cores.

out[b,o,t] = sum_{k,c} W[o, c*4+k] * x[b, c, t + k*8 - 24]

Sharding: data-parallel over batch (16 batches -> 2 per core). Each core runs
an identical program: all weights stationary in SBUF, x streamed in 512-wide
time blocks (+24 halo), 16 accumulating matmuls (4 c-chunks x 4 taps) per
(out-chunk, time-block) PSUM group, PSUM copied back via DVE and DMA'd out.

Matmuls run in bfloat16 (fp32 PSUM accumulate): 1 cycle/row streaming, and
unlike fp32/f32r the compiler-automatic Fast Weight Load path is enabled, so
the per-matmul LDWEIGHTS (97ns) hides under the previous matmul's 512-row
stream; measured steady-state cadence 216ns/MM = 512rows/2.4GHz + ~3ns NX
issue overhead, which is the HW floor (443us for 2048 MMs/core). bf16
quantization of x and W gives ~2.3e-3 relative error over the K=2048
contraction (gate 2e-2; fp8 DoubleRow measured 4e-2 on this data).

Edge optimizations (the steady state has no gaps; the NEFF preamble ~8us
and epilogue ~5us incl. a fixed 257-semaphore reset are not controllable):
- Warm-up MMs on a zeroed scratch tile keep the PE busy from the preamble
  barrier on, so the HAM activity monitor unthrottles the PE clock
  (1.2->2.4GHz after ~3.4us of sustained activity) during the DMA bootstrap
  and the first real MMs go warm as early as possible.
- x is pre-tiled on the host (halo duplicated) so every x-chunk DMA is one
  contiguous 137KB descriptor instead of 128 strided 1KB rows - the DGE
  descriptor generator, not the wire, limits bootstrap arrival.
- DMA is split across both hardware DGE queues (sync/SP + scalar/Act),
  interleaved so the first PSUM group's inputs lead both wire streams;
  steady-state output writes ride the scalar queue, inputs the sync queue.
- The very last PSUM group's drain is split across DVE+Act copies and both
  DMA queues (reusing the same tile, no extra semaphores) because the
  epilogue's reset sequence only starts once the final out-DMA completes.
"""

import ml_dtypes
import numpy as np

B = 16
C_IN = 512
C_OUT = 512
T = 8192
KW = 4
DIL = 8
PAD = (KW - 1) * DIL  # 24

N_CORES = 8
B_PER = B // N_CORES  # 2
P = 128
TBLK = 512
XW = TBLK + PAD       # 536
NT = T // TBLK        # 16
NCC = C_IN // P       # 4
NOC = C_OUT // P      # 4
N_WARM = 6

_cache = {}


def _build():
    import concourse.tile as tile
    from concourse import bacc, mybir

    nc = bacc.Bacc("TRN2", target_bir_lowering=False, debug=False,
                   num_devices=N_CORES)
    # x pre-tiled on host: [b, tb, cc, c=128, t=536] (halo duplicated)
    x = nc.dram_tensor("x", [B_PER, NT, NCC, P, XW], mybir.dt.bfloat16,
                       kind="ExternalInput").ap()
    # weights pre-arranged on host as [cc, tap, c=128, o=512]
    wt = nc.dram_tensor("wt", [NCC, KW, P, C_OUT], mybir.dt.bfloat16,
                        kind="ExternalInput").ap()
    out = nc.dram_tensor("out", [B_PER, C_OUT, T], mybir.dt.float32,
                         kind="ExternalOutput").ap()
    f32 = mybir.dt.float32
    bf16 = mybir.dt.bfloat16

    with tile.TileContext(nc) as tc:
        with tc.tile_pool(name="wpool", bufs=1) as wpool, \
             tc.tile_pool(name="xpool", bufs=4) as xpool, \
             tc.tile_pool(name="opool", bufs=4) as opool, \
             tc.tile_pool(name="pspool", bufs=4, space="PSUM") as pspool:

            # PE warm-up: zero a scratch tile and issue dependency-free MMs
            # so the PE is busy (and the HAM clock warming) during the DMA
            # bootstrap below.
            warm = wpool.tile([P, P + TBLK], bf16, name="warm", tag="warm")
            nc.vector.memset(warm[:], 0)
            wps = pspool.tile([P, TBLK], f32, name="ps", tag="ps")
            for _ in range(N_WARM):
                nc.tensor.matmul(wps[:], warm[:, :P], warm[:, P:],
                                 start=True, stop=True)

            def load_xt(b, tb, eng=None, ccs=range(NCC)):
                # One tile holds all 4 c-chunks side by side; per-chunk DMAs
                # keep arrival granularity via subtile deps.
                xt = xpool.tile([P, NCC * XW], bf16, name="xt", tag="xt")
                for cc in ccs:
                    (eng or nc.sync).dma_start(
                        xt[:, cc * XW:(cc + 1) * XW], x[b, tb, cc])
                return xt

            def rhs(xt, cc, k, o0=0, w=TBLK):
                o = cc * XW + o0 + k * DIL
                return xt[:, o: o + w]

            def lhsT(wt_cc, k, oc):
                o = k * C_OUT + oc * P
                return wt_cc[:, o: o + P]

            wtiles = [wpool.tile([P, KW * C_OUT], bf16, name=f"w{cc}",
                                 tag=f"w{cc}") for cc in range(NCC)]

            def load_w(cc, k, eng):
                eng.dma_start(wtiles[cc][:, k * C_OUT:(k + 1) * C_OUT],
                              wt[cc, k])

            # Bootstrap: the first PSUM group needs xt0-chunk0 + w0k0, so
            # they lead the two queues IN PARALLEL (x chunks on sync, w0/w1
            # on scalar); later-needed w2/w3 follow on sync.
            xt0 = load_xt(0, 0)
            for cc in (0, 1):
                for k in range(KW):
                    load_w(cc, k, nc.scalar)
            for cc in (2, 3):
                for k in range(KW):
                    load_w(cc, k, nc.sync)

            n_acc = NCC * KW
            cks = [(cc, k) for cc in range(NCC) for k in range(KW)]

            # Bootstrap block: emit MMs in weight-DMA-arrival order, fanning
            # each arriving weight chunk across the 4 oc PSUM banks, so the
            # in-order PE stream is never head-of-line blocked on a later
            # weight chunk.
            pss0 = [pspool.tile([P, TBLK], f32, name="ps", tag="ps")
                    for _ in range(NOC)]
            for ci, (cc, k) in enumerate(cks):
                for oc in range(NOC):
                    nc.tensor.matmul(
                        pss0[oc][:],
                        lhsT(wtiles[cc], k, oc),
                        rhs(xt0, cc, k),
                        start=(ci == 0),
                        stop=(ci == n_acc - 1),
                    )
            for oc in range(NOC):
                ot = opool.tile([P, TBLK], f32, name="ot", tag="ot")
                nc.vector.tensor_copy(ot[:], pss0[oc][:])
                nc.scalar.dma_start(out[0, oc * P:(oc + 1) * P, 0:TBLK],
                                    ot[:])

            H = TBLK // 2
            for b in range(B_PER):
                for tb in range(NT):
                    if b == 0 and tb == 0:
                        continue
                    final = (b == B_PER - 1 and tb == NT - 1)
                    xt = load_xt(b, tb)
                    for oc in range(NOC):
                        ps = pspool.tile([P, TBLK], f32, name="ps", tag="ps")
                        for ci, (cc, k) in enumerate(cks):
                            nc.tensor.matmul(
                                ps[:],
                                lhsT(wtiles[cc], k, oc),
                                rhs(xt, cc, k),
                                start=(ci == 0),
                                stop=(ci == n_acc - 1),
                            )
                        ot = opool.tile([P, TBLK], f32, name="ot", tag="ot")
                        orow = out[b, oc * P:(oc + 1) * P,
                                   tb * TBLK:(tb + 1) * TBLK]
                        if final and oc == NOC - 1:
                            # Split the very last drain across both copy
                            # engines and both DMA queues: the epilogue's
                            # fixed reset sequence starts only after the
                            # final out-DMA completes.
                            nc.vector.tensor_copy(ot[:, :H], ps[:, :H])
                            nc.sync.dma_start(orow[:, :H], ot[:, :H])
                            nc.scalar.copy(ot[:, H:], ps[:, H:])
                            nc.scalar.dma_start(orow[:, H:], ot[:, H:])
                        else:
                            nc.vector.tensor_copy(ot[:], ps[:])
                            # Alternate out-DMAs across both queues so
                            # neither builds a backlog that delays the
                            # end-of-NEFF barrier.
                            (nc.scalar if oc % 2 == 0 else
                             nc.sync).dma_start(orow, ot[:])

    nc.compile()
    return nc


def _get_nc():
    if "nc" not in _cache:
        _cache["nc"] = _build()
    return _cache["nc"]


def _make_in_maps(x, W):
    xb = np.ascontiguousarray(x, dtype=np.float32).astype(ml_dtypes.bfloat16)
    xpad = np.pad(xb, ((0, 0), (0, 0), (PAD, 0)))  # [B, C_IN, T+PAD]
    # pre-tile: xtl[b, tb, cc, c, j] = xpad[b, cc*128+c, tb*512 + j]
    sb, sc, st = xpad.strides
    xtv = np.lib.stride_tricks.as_strided(
        xpad, shape=(B, NT, NCC, P, XW),
        strides=(sb, st * TBLK, sc * P, sc, st))
    xtl = np.ascontiguousarray(xtv)
    w = np.ascontiguousarray(W, dtype=np.float32).reshape(C_OUT, C_IN, KW)
    # wt[cc, k, c, o] = W[o, (cc*128+c)*KW + k]
    wt = np.transpose(w.reshape(C_OUT, NCC, P, KW),
                      (1, 3, 2, 0)).astype(ml_dtypes.bfloat16).copy()
    return [{"x": np.ascontiguousarray(xtl[i * B_PER:(i + 1) * B_PER]),
             "wt": wt} for i in range(N_CORES)]


def kernel(x, W):
    from concourse.bass_utils import run_bass_kernel_spmd

    nc = _get_nc()
    in_maps = _make_in_maps(x, W)
    res = run_bass_kernel_spmd(nc, in_maps, list(range(N_CORES)))
    return np.concatenate([r["out"] for r in res.results], axis=0)
